# revision 2
# baseline (speedup 1.0000x reference)
"""Trainium2 Bass kernel for the 2-layer LSTM (H=51 -> H=1) over T=2048 steps.

Data-parallel over batch: 8 cores x 128 batch (batch on the free dim).
Hardware For_i loop over T/CH chunks of CH unrolled steps keeps the program
tiny (per-call BIR->NEFF assembly + NEFF load dominate warm wall time).

Per core/step one PSUM tile P (52,512) holds all gate pre-activations for
BOTH layers: gate blocks I,F,G,O in 128-col blocks; unit index m: m=0 is
layer-2 (so its h2/c2 sit at partition 0 -- compute APs must start at a
partition multiple of 32), m=1+u is layer-1 unit u. tanh(z)=2*sigmoid(2z)-1
with the x2 folded into the G-block weights, so ONE Sigmoid covers all four
gates. x_t enters as state row 64 (32-aligned) via an ACT copy from a flat
fp16 chunk stripe; A row 64 = W_ih1. Layer 2 lags one step; its step-0
garbage is killed exactly by initializing the h2 state row so the step-0
layer-2 i-gate (or g-gate) zeroes c2/h2. y rows collect in an fp16 stripe
DMA'd out once per chunk.
"""

import numpy as np

H = 51
B = 128
NCORES = 8
N_FULL = 1024
T_FULL = 2048
CH = 16          # time steps per For_i chunk
KR = 65          # state rows: 0 h2, 1:52 h1, 52 const-1, 53:64 pad, 64 x


def pack_weights(W_ih1, W_hh1, b_ih1, b_hh1, W_ih2, W_hh2, b_ih2, b_hh2):
    """lhsT pack (KR, 208). K rows: 0 h2, 1:52 h1, 52 const-1(bias), 64 x.
    M cols per 52-block: 0 layer-2, 1:52 layer-1. G block x2 (tanh fold)."""
    def block(l1_rows, l2_row, scale):
        L = np.zeros((KR, 52), np.float32)
        L[0, 0] = W_hh2[l2_row, 0]
        L[1:52, 0] = W_ih2[l2_row, :]
        L[52, 0] = b_ih2[l2_row] + b_hh2[l2_row]
        L[1:52, 1:52] = W_hh1[l1_rows, :].T
        L[52, 1:52] = b_ih1[l1_rows] + b_hh1[l1_rows]
        L[64, 1:52] = W_ih1[l1_rows, 0]
        return L * scale

    A = np.concatenate([
        block(slice(0, 51), 0, 1.0),      # I
        block(slice(51, 102), 1, 1.0),    # F
        block(slice(102, 153), 2, 2.0),   # G* (x2)
        block(slice(153, 204), 3, 1.0),   # O
    ], axis=1)                            # (KR, 208)

    # h2-state init v so that step-0 layer-2 output is exactly (near) zero:
    # prefer forcing i2 pre-act to -40 (sig ~ 4e-18); fall back to g2 = 0.
    b_i2 = float(b_ih2[0] + b_hh2[0])
    w_i2 = float(W_hh2[0, 0])
    b_g2 = float(b_ih2[2] + b_hh2[2])
    w_g2 = float(W_hh2[2, 0])
    if abs(w_i2) >= abs(w_g2):
        assert abs(w_i2) > 1e-12, "degenerate W_hh2"
        v = (-40.0 - b_i2) / w_i2
    else:
        v = -b_g2 / w_g2
    return A, float(v)


def build_program(v, T=T_FULL, debug=False):
    import concourse.bass as bass
    import concourse.tile as tile
    from concourse import bacc, mybir
    from concourse.bass import ds

    dt = mybir.dt.float32
    dth = mybir.dt.float16
    dt8 = mybir.dt.float8e4
    nc = bacc.Bacc("TRN2", target_bir_lowering=False, debug=debug)

    nxch = T // CH
    assert nxch * CH == T and CH % 2 == 0
    X_d = nc.dram_tensor("X", [nxch, CH * B], dt8, kind="ExternalInput")
    Y_d = nc.dram_tensor("Y", [nxch + 1, CH * B], dth, kind="ExternalOutput")
    A_d = nc.dram_tensor("A", [KR, 208], dt, kind="ExternalInput")

    SIG = mybir.ActivationFunctionType.Sigmoid
    TANH = mybir.ActivationFunctionType.Tanh
    COPY = mybir.ActivationFunctionType.Copy
    MUL = mybir.AluOpType.mult
    SUB = mybir.AluOpType.subtract

    with tile.TileContext(nc) as tc:
        with (
            tc.tile_pool(name="wts", bufs=1) as wpool,
            tc.tile_pool(name="state", bufs=1) as stpool,
            tc.tile_pool(name="xf", bufs=2) as xfpool,
            tc.tile_pool(name="sg", bufs=2) as spool,
            tc.tile_pool(name="tmp", bufs=2) as tpool,
            tc.tile_pool(name="ps", bufs=2, space=bass.MemorySpace.PSUM) as ppool,
        ):
            A = wpool.tile([KR, 208], dt, tag="A")
            nc.sync.dma_start(A[:], A_d[:])

            ones = wpool.tile([1, B], dt, tag="ones")
            nc.vector.memset(ones[:], 1.0)

            # state: R parity pair (KR,B); cc rows: 0 c2, 1:52 c1
            R0 = stpool.tile([KR, B], dt, tag="R0")
            R1 = stpool.tile([KR, B], dt, tag="R1")
            Rp = [R0, R1]
            cc = stpool.tile([52, B], dt, tag="cc")
            ystripe = stpool.tile([1, (CH + 1) * B], dth, tag="ys")
            nc.vector.memset(R0[:], 0.0)
            nc.vector.memset(R1[:], 0.0)
            nc.vector.memset(R0[0:1, :], v)  # layer-2 step-0 zeroing
            nc.sync.dma_start(R0[52:53, :], ones[:])
            nc.sync.dma_start(R1[52:53, :], ones[:])
            nc.vector.memset(cc[:], 0.0)
            nc.vector.memset(ystripe[:], 0.0)

            def step(j, with_h1):
                Rin = Rp[j % 2]
                Rout = Rp[(j + 1) % 2]
                P = ppool.tile([52, 4 * B], dt, tag="P")
                for g in range(4):
                    nc.tensor.matmul(P[:, g * B:(g + 1) * B],
                                     A[:, g * 52:(g + 1) * 52],
                                     Rin[:], start=True, stop=True)
                S = spool.tile([52, 4 * B], dt, tag="S")
                nc.scalar.activation(S[:], P[:], SIG)
                s_I = S[:, 0:B]
                s_F = S[:, B:2 * B]
                s_G = S[:, 2 * B:3 * B]
                s_O = S[:, 3 * B:4 * B]

                m = tpool.tile([52, B], dt, tag="m")
                t1 = tpool.tile([52, B], dt, tag="t1")
                t2 = tpool.tile([52, B], dt, tag="t2")
                tau = tpool.tile([52, B], dt, tag="tau")
                nc.vector.tensor_mul(m[:], s_I, s_G)
                nc.vector.scalar_tensor_tensor(t1[:], m[:], 2.0, s_I,
                                               op0=MUL, op1=SUB)
                nc.vector.tensor_mul(t2[:], s_F, cc[:])
                nc.vector.tensor_add(cc[:], t1[:], t2[:])
                nc.scalar.activation(tau[:], cc[:], TANH)
                if with_h1:
                    nc.vector.tensor_mul(Rout[0:52, :], s_O, tau[:])
                # y value h2 produced this step -> stripe slot j%CH + 1
                slot = (j % CH) + 1
                nc.vector.tensor_mul(ystripe[0:1, slot * B:(slot + 1) * B],
                                     s_O[0:1, :], tau[0:1, :])

            with tc.For_i(0, nxch, 1) as k:
                Xf = xfpool.tile([1, CH * B], dt8, tag="Xf")
                nc.sync.dma_start(Xf[:], X_d[ds(k, 1), :])
                for j in range(CH):
                    # x_t into state row 64 (fp16 -> f32) before the matmuls
                    nc.scalar.activation(Rp[j % 2][64:65, :],
                                         Xf[0:1, j * B:(j + 1) * B], COPY)
                    step(j, with_h1=True)
                nc.sync.dma_start(Y_d[ds(k, 1), :], ystripe[0:1, 0:CH * B])
                nc.vector.tensor_copy(ystripe[0:1, 0:B],
                                      ystripe[0:1, CH * B:(CH + 1) * B])

            # epilogue step s=T: layer-2 finishes time T-1 (x row is stale;
            # A row 64 has 0 in every layer-2 column so layer 2 is
            # unaffected, and the garbage h1 is never used).
            step(T, with_h1=False)
            nc.sync.dma_start(Y_d[nxch:nxch + 1, 0:2 * B],
                              ystripe[0:1, 0:2 * B])

    nc.compile()
    return nc


def _prep_inputs(stimulus, pk_A, T):
    from concourse import mybir
    f8 = mybir.dt.np(mybir.dt.float8e4)
    nxch = T // CH
    xa = stimulus.astype(f8)  # (N, T)
    in_maps = []
    for c in range(NCORES):
        xc = np.ascontiguousarray(xa[c * B:(c + 1) * B, :].T)  # (T, B)
        in_maps.append({"X": xc.reshape(nxch, CH * B), "A": pk_A})
    return in_maps


def _post_outputs(res, T):
    nxch = T // CH
    cols = []
    for c in range(NCORES):
        y16 = res.results[c]["Y"]                  # (nxch+1, CH*B) fp16
        flat = y16[0:nxch].reshape(T, B)
        tail = y16[nxch, 0:2 * B].reshape(2, B)
        ypad = np.concatenate([flat, tail], axis=0)  # padded rows 0..T+1
        cols.append(ypad[2:T + 2].astype(np.float32).T)  # (B, T)
    return np.ascontiguousarray(np.concatenate(cols, axis=0))  # (N, T)


_PROG_CACHE = {}


def kernel(stimulus, W_ih1, W_hh1, b_ih1, b_hh1, W_ih2, W_hh2, b_ih2, b_hh2):
    from concourse.bass_utils import run_bass_kernel_spmd

    stimulus = np.asarray(stimulus, np.float32)
    args = [np.asarray(a, np.float32)
            for a in (W_ih1, W_hh1, b_ih1, b_hh1, W_ih2, W_hh2, b_ih2, b_hh2)]
    N, T = stimulus.shape
    assert N == N_FULL
    pk_A, v = pack_weights(*args)
    key = (T, v)
    nc = _PROG_CACHE.get(key)
    if nc is None:
        nc = build_program(v, T=T)
        _PROG_CACHE[key] = nc
    in_maps = _prep_inputs(stimulus, pk_A, T)
    res = run_bass_kernel_spmd(nc, in_maps, list(range(NCORES)))
    return _post_outputs(res, T)


# revision 3
# speedup vs baseline: 1.4016x; 1.4016x over previous
"""Trainium2 Bass kernel for the 2-layer LSTM (H=51 -> H=1) over T=2048 steps.

Data-parallel over batch: 8 cores x 128 batch (batch on the free dim).
Hardware For_i loop over T/CH chunks of CH unrolled steps keeps the program
tiny (per-call BIR->NEFF assembly + NEFF load dominate warm wall time).

Per core/step one PSUM tile P (52,512) holds all gate pre-activations for
BOTH layers: gate blocks I,F,G,O in 128-col blocks; unit index m: m=0 is
layer-2 (so its h2/c2 sit at partition 0 -- compute APs must start at a
partition multiple of 32), m=1+u is layer-1 unit u. tanh(z)=2*sigmoid(2z)-1
with the x2 folded into the G-block weights, so ONE Sigmoid covers all four
gates. x_t enters as state row 64 (32-aligned) via an ACT copy from a flat
fp16 chunk stripe; A row 64 = W_ih1. Layer 2 lags one step; its step-0
garbage is killed exactly by initializing the h2 state row so the step-0
layer-2 i-gate (or g-gate) zeroes c2/h2. y rows collect in an fp16 stripe
DMA'd out once per chunk.
"""

import numpy as np

H = 51
B = 128
NCORES = 8
N_FULL = 1024
T_FULL = 2048
CH = 16          # time steps per For_i chunk
KR = 65          # state rows: 0 h2, 1:52 h1, 52 const-1, 53:64 pad, 64 x


def pack_weights(W_ih1, W_hh1, b_ih1, b_hh1, W_ih2, W_hh2, b_ih2, b_hh2):
    """lhsT pack (KR, 208). K rows: 0 h2, 1:52 h1, 52 const-1(bias), 64 x.
    M cols per 52-block: 0 layer-2, 1:52 layer-1. G block x2 (tanh fold)."""
    def block(l1_rows, l2_row, scale):
        L = np.zeros((KR, 52), np.float32)
        L[0, 0] = W_hh2[l2_row, 0]
        L[1:52, 0] = W_ih2[l2_row, :]
        L[52, 0] = b_ih2[l2_row] + b_hh2[l2_row]
        L[1:52, 1:52] = W_hh1[l1_rows, :].T
        L[52, 1:52] = b_ih1[l1_rows] + b_hh1[l1_rows]
        L[64, 1:52] = W_ih1[l1_rows, 0]
        return L * scale

    A = np.concatenate([
        block(slice(0, 51), 0, 1.0),      # I
        block(slice(51, 102), 1, 1.0),    # F
        block(slice(102, 153), 2, 2.0),   # G* (x2)
        block(slice(153, 204), 3, 1.0),   # O
    ], axis=1)                            # (KR, 208)

    # h2-state init v so that step-0 layer-2 output is exactly (near) zero:
    # prefer forcing i2 pre-act to -40 (sig ~ 4e-18); fall back to g2 = 0.
    b_i2 = float(b_ih2[0] + b_hh2[0])
    w_i2 = float(W_hh2[0, 0])
    b_g2 = float(b_ih2[2] + b_hh2[2])
    w_g2 = float(W_hh2[2, 0])
    if abs(w_i2) >= abs(w_g2):
        assert abs(w_i2) > 1e-12, "degenerate W_hh2"
        v = (-40.0 - b_i2) / w_i2
    else:
        v = -b_g2 / w_g2
    return A, float(v)


def build_program(v, T=T_FULL, debug=False):
    import concourse.bass as bass
    import concourse.tile as tile
    from concourse import bacc, mybir
    from concourse.bass import ds

    dt = mybir.dt.float32
    dth = mybir.dt.float16
    nc = bacc.Bacc("TRN2", target_bir_lowering=False, debug=debug)

    nxch = T // CH
    assert nxch * CH == T and CH % 2 == 0
    X_d = nc.dram_tensor("X", [nxch, CH * B], dth, kind="ExternalInput")
    Y_d = nc.dram_tensor("Y", [nxch + 1, CH * B], dth, kind="ExternalOutput")
    A_d = nc.dram_tensor("A", [KR, 208], dt, kind="ExternalInput")

    SIG = mybir.ActivationFunctionType.Sigmoid
    TANH = mybir.ActivationFunctionType.Tanh
    COPY = mybir.ActivationFunctionType.Copy
    MUL = mybir.AluOpType.mult
    SUB = mybir.AluOpType.subtract

    with tile.TileContext(nc) as tc:
        with (
            tc.tile_pool(name="wts", bufs=1) as wpool,
            tc.tile_pool(name="state", bufs=1) as stpool,
            tc.tile_pool(name="xf", bufs=2) as xfpool,
            tc.tile_pool(name="sg", bufs=2) as spool,
            tc.tile_pool(name="tmp", bufs=2) as tpool,
            tc.tile_pool(name="ps", bufs=2, space=bass.MemorySpace.PSUM) as ppool,
        ):
            A = wpool.tile([KR, 208], dt, tag="A")
            nc.sync.dma_start(A[:], A_d[:])

            ones = wpool.tile([1, B], dt, tag="ones")
            nc.vector.memset(ones[:], 1.0)

            # state: R parity pair (KR,B); cc rows: 0 c2, 1:52 c1
            R0 = stpool.tile([KR, B], dt, tag="R0")
            R1 = stpool.tile([KR, B], dt, tag="R1")
            Rp = [R0, R1]
            cc = stpool.tile([52, B], dt, tag="cc")
            ystripe = stpool.tile([1, (CH + 1) * B], dth, tag="ys")
            nc.vector.memset(R0[:], 0.0)
            nc.vector.memset(R1[:], 0.0)
            nc.vector.memset(R0[0:1, :], v)  # layer-2 step-0 zeroing
            nc.sync.dma_start(R0[52:53, :], ones[:])
            nc.sync.dma_start(R1[52:53, :], ones[:])
            nc.vector.memset(cc[:], 0.0)
            nc.vector.memset(ystripe[:], 0.0)

            def step(j, with_h1):
                Rin = Rp[j % 2]
                Rout = Rp[(j + 1) % 2]
                P = ppool.tile([52, 4 * B], dt, tag="P")
                for g in range(4):
                    nc.tensor.matmul(P[:, g * B:(g + 1) * B],
                                     A[:, g * 52:(g + 1) * 52],
                                     Rin[:], start=True, stop=True)
                S = spool.tile([52, 4 * B], dt, tag="S")
                nc.scalar.activation(S[:], P[:], SIG)
                s_I = S[:, 0:B]
                s_F = S[:, B:2 * B]
                s_G = S[:, 2 * B:3 * B]
                s_O = S[:, 3 * B:4 * B]

                m = tpool.tile([52, B], dt, tag="m")
                t1 = tpool.tile([52, B], dt, tag="t1")
                t2 = tpool.tile([52, B], dt, tag="t2")
                tau = tpool.tile([52, B], dt, tag="tau")
                nc.vector.tensor_mul(m[:], s_I, s_G)
                nc.vector.scalar_tensor_tensor(t1[:], m[:], 2.0, s_I,
                                               op0=MUL, op1=SUB)
                nc.vector.tensor_mul(t2[:], s_F, cc[:])
                nc.vector.tensor_add(cc[:], t1[:], t2[:])
                nc.scalar.activation(tau[:], cc[:], TANH)
                if with_h1:
                    nc.vector.tensor_mul(Rout[0:52, :], s_O, tau[:])
                # y value h2 produced this step -> stripe slot j%CH + 1
                slot = (j % CH) + 1
                nc.vector.tensor_mul(ystripe[0:1, slot * B:(slot + 1) * B],
                                     s_O[0:1, :], tau[0:1, :])

            with tc.For_i(0, nxch, 1) as k:
                Xf = xfpool.tile([1, CH * B], dth, tag="Xf")
                nc.sync.dma_start(Xf[:], X_d[ds(k, 1), :])
                for j in range(CH):
                    # x_t into state row 64 (fp16 -> f32) before the matmuls
                    nc.scalar.activation(Rp[j % 2][64:65, :],
                                         Xf[0:1, j * B:(j + 1) * B], COPY)
                    step(j, with_h1=True)
                nc.sync.dma_start(Y_d[ds(k, 1), :], ystripe[0:1, 0:CH * B])
                nc.vector.tensor_copy(ystripe[0:1, 0:B],
                                      ystripe[0:1, CH * B:(CH + 1) * B])

            # epilogue step s=T: layer-2 finishes time T-1 (x row is stale;
            # A row 64 has 0 in every layer-2 column so layer 2 is
            # unaffected, and the garbage h1 is never used).
            step(T, with_h1=False)
            nc.sync.dma_start(Y_d[nxch:nxch + 1, 0:2 * B],
                              ystripe[0:1, 0:2 * B])

    nc.compile()
    return nc


def _prep_inputs(stimulus, pk_A, T):
    nxch = T // CH
    xa = stimulus.astype(np.float16)  # (N, T)
    in_maps = []
    for c in range(NCORES):
        xc = np.ascontiguousarray(xa[c * B:(c + 1) * B, :].T)  # (T, B)
        in_maps.append({"X": xc.reshape(nxch, CH * B), "A": pk_A})
    return in_maps


def _post_outputs(res, T):
    nxch = T // CH
    cols = []
    for c in range(NCORES):
        y16 = res.results[c]["Y"]                  # (nxch+1, CH*B) fp16
        flat = y16[0:nxch].reshape(T, B)
        tail = y16[nxch, 0:2 * B].reshape(2, B)
        ypad = np.concatenate([flat, tail], axis=0)  # padded rows 0..T+1
        cols.append(ypad[2:T + 2].astype(np.float32).T)  # (B, T)
    return np.ascontiguousarray(np.concatenate(cols, axis=0))  # (N, T)


_PROG_CACHE = {}


def kernel(stimulus, W_ih1, W_hh1, b_ih1, b_hh1, W_ih2, W_hh2, b_ih2, b_hh2):
    from concourse.bass_utils import run_bass_kernel_spmd

    stimulus = np.asarray(stimulus, np.float32)
    args = [np.asarray(a, np.float32)
            for a in (W_ih1, W_hh1, b_ih1, b_hh1, W_ih2, W_hh2, b_ih2, b_hh2)]
    N, T = stimulus.shape
    assert N == N_FULL
    pk_A, v = pack_weights(*args)
    key = (T, v)
    nc = _PROG_CACHE.get(key)
    if nc is None:
        nc = build_program(v, T=T)
        _PROG_CACHE[key] = nc
    in_maps = _prep_inputs(stimulus, pk_A, T)
    res = run_bass_kernel_spmd(nc, in_maps, list(range(NCORES)))
    return _post_outputs(res, T)


# revision 4
# speedup vs baseline: 2.4164x; 1.7240x over previous
"""Trainium2 Bass kernel for the 2-layer LSTM (H=51 -> H=1) over T=2048 steps.

Data-parallel over batch: 8 cores x 128 batch (batch on the free dim).
Hardware For_i loop over T/CH chunks of CH unrolled steps keeps the program
tiny (per-call BIR->NEFF assembly + NEFF load dominate warm wall time).

Per core/step one PSUM tile P (52,512) holds all gate pre-activations for
BOTH layers: gate blocks I,F,G,O in 128-col blocks; unit index m: m=0 is
layer-2 (so its h2/c2 sit at partition 0 -- compute APs must start at a
partition multiple of 32), m=1+u is layer-1 unit u. tanh(z)=2*sigmoid(2z)-1
with the x2 folded into the G-block weights, so ONE Sigmoid covers all four
gates. x_t enters as state row 64 (32-aligned) via an ACT copy from a flat
fp16 chunk stripe; A row 64 = W_ih1. Layer 2 lags one step; its step-0
garbage is killed exactly by initializing the h2 state row so the step-0
layer-2 i-gate (or g-gate) zeroes c2/h2. y rows collect in an fp16 stripe
DMA'd out once per chunk.
"""

import numpy as np

H = 51
B = 128
NCORES = 8
N_FULL = 1024
T_FULL = 2048
CH = 16          # time steps per For_i chunk
KR = 65          # state rows: 0 h2, 1:52 h1, 52 const-1, 53:64 pad, 64 x


def pack_weights(W_ih1, W_hh1, b_ih1, b_hh1, W_ih2, W_hh2, b_ih2, b_hh2):
    """lhsT pack (KR, 208). K rows: 0 h2, 1:52 h1, 52 const-1(bias), 64 x.
    M cols per 52-block: 0 layer-2, 1:52 layer-1. G block x2 (tanh fold)."""
    def block(l1_rows, l2_row, scale):
        L = np.zeros((KR, 52), np.float32)
        L[0, 0] = W_hh2[l2_row, 0]
        L[1:52, 0] = W_ih2[l2_row, :]
        L[52, 0] = b_ih2[l2_row] + b_hh2[l2_row]
        L[1:52, 1:52] = W_hh1[l1_rows, :].T
        L[52, 1:52] = b_ih1[l1_rows] + b_hh1[l1_rows]
        L[64, 1:52] = W_ih1[l1_rows, 0]
        return L * scale

    A = np.concatenate([
        block(slice(0, 51), 0, 1.0),      # I
        block(slice(51, 102), 1, 1.0),    # F
        block(slice(102, 153), 2, 2.0),   # G* (x2)
        block(slice(153, 204), 3, 1.0),   # O
    ], axis=1)                            # (KR, 208)

    # h2-state init v so that step-0 layer-2 output is exactly (near) zero:
    # prefer forcing i2 pre-act to -40 (sig ~ 4e-18); fall back to g2 = 0.
    b_i2 = float(b_ih2[0] + b_hh2[0])
    w_i2 = float(W_hh2[0, 0])
    b_g2 = float(b_ih2[2] + b_hh2[2])
    w_g2 = float(W_hh2[2, 0])
    if abs(w_i2) >= abs(w_g2):
        assert abs(w_i2) > 1e-12, "degenerate W_hh2"
        v = (-40.0 - b_i2) / w_i2
    else:
        v = -b_g2 / w_g2
    return A, float(v)


def build_program(v, T=T_FULL, debug=False):
    import concourse.bass as bass
    import concourse.tile as tile
    from concourse import bacc, mybir
    from concourse.bass import ds

    dt = mybir.dt.float32
    dth = mybir.dt.float16
    nc = bacc.Bacc("TRN2", target_bir_lowering=False, debug=debug)

    nxch = T // CH
    assert nxch * CH == T and CH % 2 == 0
    X_d = nc.dram_tensor("X", [nxch, CH * B], dth, kind="ExternalInput")
    Y_d = nc.dram_tensor("Y", [nxch + 1, CH * B], dth, kind="ExternalOutput")
    A_d = nc.dram_tensor("A", [KR, 208], dt, kind="ExternalInput")

    SIG = mybir.ActivationFunctionType.Sigmoid
    TANH = mybir.ActivationFunctionType.Tanh
    COPY = mybir.ActivationFunctionType.Copy
    MUL = mybir.AluOpType.mult
    SUB = mybir.AluOpType.subtract

    with tile.TileContext(nc) as tc:
        with (
            tc.tile_pool(name="wts", bufs=1) as wpool,
            tc.tile_pool(name="state", bufs=1) as stpool,
            tc.tile_pool(name="xf", bufs=2) as xfpool,
            tc.tile_pool(name="sg", bufs=2) as spool,
            tc.tile_pool(name="tmp", bufs=2) as tpool,
            tc.tile_pool(name="ps", bufs=2, space=bass.MemorySpace.PSUM) as ppool,
        ):
            A = wpool.tile([KR, 208], dt, tag="A")
            nc.sync.dma_start(A[:], A_d[:])

            ones = wpool.tile([1, B], dt, tag="ones")
            nc.vector.memset(ones[:], 1.0)

            # state: R parity pair (KR,B); cc rows: 0 c2, 1:52 c1
            R0 = stpool.tile([KR, B], dt, tag="R0")
            R1 = stpool.tile([KR, B], dt, tag="R1")
            Rp = [R0, R1]
            cc = stpool.tile([52, B], dt, tag="cc")
            ystripe = stpool.tile([1, (CH + 1) * B], dth, tag="ys")
            nc.vector.memset(R0[:], 0.0)
            nc.vector.memset(R1[:], 0.0)
            nc.vector.memset(R0[0:1, :], v)  # layer-2 step-0 zeroing
            nc.sync.dma_start(R0[52:53, :], ones[:])
            nc.sync.dma_start(R1[52:53, :], ones[:])
            nc.vector.memset(cc[:], 0.0)
            nc.vector.memset(ystripe[:], 0.0)

            def step(j, with_h1):
                Rin = Rp[j % 2]
                Rout = Rp[(j + 1) % 2]
                P = ppool.tile([52, 4 * B], dt, tag="P")
                for g in range(4):
                    nc.tensor.matmul(P[:, g * B:(g + 1) * B],
                                     A[:, g * 52:(g + 1) * 52],
                                     Rin[:], start=True, stop=True)
                S = spool.tile([52, 4 * B], dt, tag="S")
                nc.scalar.activation(S[:], P[:], SIG)
                s_I = S[:, 0:B]
                s_F = S[:, B:2 * B]
                s_G = S[:, 2 * B:3 * B]
                s_O = S[:, 3 * B:4 * B]

                m = tpool.tile([52, B], dt, tag="m")
                t1 = tpool.tile([52, B], dt, tag="t1")
                t2 = tpool.tile([52, B], dt, tag="t2")
                tau = tpool.tile([52, B], dt, tag="tau")
                nc.vector.tensor_mul(m[:], s_I, s_G)
                nc.vector.scalar_tensor_tensor(t1[:], m[:], 2.0, s_I,
                                               op0=MUL, op1=SUB)
                nc.vector.tensor_mul(t2[:], s_F, cc[:])
                nc.vector.tensor_add(cc[:], t1[:], t2[:])
                nc.scalar.activation(tau[:], cc[:], TANH)
                if with_h1:
                    nc.vector.tensor_mul(Rout[0:52, :], s_O, tau[:])
                # y value h2 produced this step -> stripe slot j%CH + 1
                slot = (j % CH) + 1
                nc.vector.tensor_mul(ystripe[0:1, slot * B:(slot + 1) * B],
                                     s_O[0:1, :], tau[0:1, :])

            with tc.For_i(0, nxch, 1) as k:
                Xf = xfpool.tile([1, CH * B], dth, tag="Xf")
                nc.sync.dma_start(Xf[:], X_d[ds(k, 1), :])
                for j in range(CH):
                    # x_t into state row 64 (fp16 -> f32) before the matmuls
                    nc.scalar.activation(Rp[j % 2][64:65, :],
                                         Xf[0:1, j * B:(j + 1) * B], COPY)
                    step(j, with_h1=True)
                nc.sync.dma_start(Y_d[ds(k, 1), :], ystripe[0:1, 0:CH * B])
                nc.vector.tensor_copy(ystripe[0:1, 0:B],
                                      ystripe[0:1, CH * B:(CH + 1) * B])

            # epilogue step s=T: layer-2 finishes time T-1 (x row is stale;
            # A row 64 has 0 in every layer-2 column so layer 2 is
            # unaffected, and the garbage h1 is never used).
            step(T, with_h1=False)
            nc.sync.dma_start(Y_d[nxch:nxch + 1, 0:2 * B],
                              ystripe[0:1, 0:2 * B])

    nc.compile()
    return nc


def _prep_inputs(stimulus, pk_A, T):
    nxch = T // CH
    xa = stimulus.astype(np.float16)  # (N, T)
    in_maps = []
    for c in range(NCORES):
        xc = np.ascontiguousarray(xa[c * B:(c + 1) * B, :].T)  # (T, B)
        in_maps.append({"X": xc.reshape(nxch, CH * B), "A": pk_A})
    return in_maps


def _post_outputs(res, T):
    nxch = T // CH
    out = np.empty((N_FULL, T), np.float32)
    for c in range(NCORES):
        y16 = res.results[c]["Y"]                  # (nxch+1, CH*B) fp16
        flat = y16[0:nxch].reshape(T, B)           # padded rows 0..T-1
        tail = y16[nxch, 0:2 * B].reshape(2, B)    # padded rows T, T+1
        rows = slice(c * B, (c + 1) * B)
        out[rows, 0:T - 2] = flat[2:T].T           # y[t] = padded row t+2
        out[rows, T - 2:T] = tail.T
    return out


_PROG_CACHE = {}


def kernel(stimulus, W_ih1, W_hh1, b_ih1, b_hh1, W_ih2, W_hh2, b_ih2, b_hh2):
    from concourse.bass_utils import run_bass_kernel_spmd

    stimulus = np.asarray(stimulus, np.float32)
    args = [np.asarray(a, np.float32)
            for a in (W_ih1, W_hh1, b_ih1, b_hh1, W_ih2, W_hh2, b_ih2, b_hh2)]
    N, T = stimulus.shape
    assert N == N_FULL
    pk_A, v = pack_weights(*args)
    key = (T, v)
    nc = _PROG_CACHE.get(key)
    if nc is None:
        nc = build_program(v, T=T)
        _PROG_CACHE[key] = nc
    in_maps = _prep_inputs(stimulus, pk_A, T)
    res = run_bass_kernel_spmd(nc, in_maps, list(range(NCORES)))
    return _post_outputs(res, T)
